# revision 41
# baseline (speedup 1.0000x reference)
"""Grouped per-channel Linear + ReLU on 8 TRN2 NeuronCores.

Problem: out[b,c,e] = relu(sum_s x[b,s,c] * W[c,s,e] + bias[c,e])
  x: (256, 2048, 32) f32, W: (32, 2048, 2048) f32, bias: (32, 2048) f32
  out: (256, 32, 2048) f32

Sharding: expert/channel parallel - core i computes channels [4i, 4i+4).
Each core runs 4 independent GEMMs of (256x2048)@(2048x2048) with the
contraction dim S on SBUF partitions.

Precision: the PE runs fp8 E3M4 at full bf16 rate and the two operand
dtypes of a matmul are independent (both mixed directions verified on
HW). W k-tiles are fp16 or fp8 per a schedule below (46 of 64 fp8,
host pre-scaled by 2^9 into E3M4's normal range); x (stationary
operand) is fp8 on channels 0-1 (pre-scaled by 2) and fp16 on 2-3. The
eviction activation undoes the exact power-of-two scaling. Measured
rel l2 error ~1.5e-2 against the 2e-2 gate.

Why mixed precision, and why this placement: two NeuronCores share one
716 GB/s HBM stack, and at kernel start all 8 cores pull their W
streams at once, so the first ~60 us are chip-bandwidth-starved: W
k-tiles arrive slower than the PE's ~294 GB/s consumption, the PE gaps,
and each >3.4 us gap re-throttles the HAM clock gate to 1.2 GHz. The
fp8 k-tiles are therefore concentrated in the contended window -
channel 0 (k2-15), all of channel 1, channel 2 (k10-15) - halving the
bytes exactly where bandwidth is scarce, plus channel 3's tail (k12-15)
which coincides with x-prefetch/output bursts. Steady state is PE-bound
at the 216 ns/matmul roofline (512 N=512 matmuls = 110.5 us/core).

Schedule: W chunks (2 k-tiles; singles for the channel-0 ramp) are
E-half split across BOTH HWDGE rings, so each half unlocks its et
matmuls independently and the PE sees work every ~0.25-0.5 MB of
delivery. Channel-0 k6-7 ride the SWDGE ring (third parallel channel
during the head); ~48 zero warmup matmuls keep the PE busy before the
first data lands so the HAM reaches 8/8 early. x slabs prefetch one
channel ahead on SWDGE, gated behind a mid-channel W chunk; outputs for
channels 0-2 leave on SWDGE; the last channel's leave eagerly per
512-col subtile on all three rings. Eviction: VectorE adds the
broadcast bias, ScalarE applies ReLU + 2^-9 unscale + fp16 cast. The
LAST channel instead seeds its PSUM banks with the bias via K=1
ones-row matmuls, so its eviction is one fused op per bank, alternated
across ScalarE/VectorE - this halves the serial evict chain that forms
the kernel tail after the final matmul.
"""

import os
import sys

for _p in ("/opt/trn_rl_repo", "/root/.axon_site/_ro/trn_rl_repo"):
    if os.path.isdir(_p) and _p not in sys.path:
        sys.path.insert(0, _p)

import numpy as np
import ml_dtypes

import concourse.bacc as bacc
import concourse.mybir as mybir
from concourse import tile
from concourse.bass_utils import run_bass_kernel_spmd
from concourse.tile_rust import add_dep_helper

B, S, C, E = 256, 2048, 32, 2048
NCORES = 8
CPC = C // NCORES          # channels per core = 4
P = 128
KT = S // P                # 16 k-tiles
NBT = B // P               # 2 batch tiles
FREE = 512                 # matmul moving free dim (one PSUM bank of f32)
NET = E // FREE            # 4 e-tiles
KC = 2                     # k-tiles per W DMA chunk
WBUFS = 10                 # fp16 W chunk lookahead
W8BUFS = 12                # fp8 W chunk lookahead
NWARM = 48                 # PE warmup matmuls during the DMA head
WSCALE = 512.0             # host W/bias pre-scale (2^9), undone at evict
XSCALE = 2.0               # extra x pre-scale for the fp8-x channels 0-1

# per-channel fp8 k-tiles: concentrated in the bandwidth-starved first
# ~60us (all of ch0 after the first two tiles, first 10 tiles of ch1)
# plus the channel tails that coincide with prefetch/output bursts.
K8 = {
    0: frozenset(range(2, 16)),
    1: frozenset(range(0, 16)),
    2: frozenset({0, 1, 2, 3, 4, 5, 10, 11, 12, 13, 14, 15}),
    3: frozenset({12, 13, 14, 15}),
}

_nc_cache = {}


def _chunks(c):
    """[(k0, nkt, is8)] covering k 0..15; singles below k8 on channel 0."""
    out = []
    k = 0
    while k < KT:
        is8 = k in K8[c]
        n = 1
        if (
            not (c == 0 and k < 8)
            and k + 1 < KT
            and ((k + 1) in K8[c]) == is8
        ):
            n = KC
        out.append((k, n, is8))
        k += n
    return out


def _build():
    f16 = mybir.dt.float16
    f32 = mybir.dt.float32
    f8 = mybir.dt.float8e3
    nc = bacc.Bacc(None, target_bir_lowering=False)
    # xt[c, p, k*B + b] = x[b, k*P + p, c] : contiguous per partition.
    # Channels 0-1 (the bandwidth-contended head) carry x in fp8 e3m4
    # (scaled by 2 so |x| <= ~11 sits in E3M4's normal range); 2-3 fp16.
    xt8 = nc.dram_tensor("xt8", [2, P, KT * B], f8, kind="ExternalInput")
    xt16 = nc.dram_tensor(
        "xt16", [CPC - 2, P, KT * B], f16, kind="ExternalInput"
    )
    # w16[c, p, j, e] / w8[c, p, j, e]: j-th fp16/fp8 k-tile of channel c
    # (k-tiles packed per dtype in k order), value WSCALE * W[c, k*P+p, e]
    n16 = sum(KT - len(K8[c]) for c in range(CPC))
    n8 = sum(len(K8[c]) for c in range(CPC))
    w16 = nc.dram_tensor("w16", [n16, P, E], f16, kind="ExternalInput")
    w8 = nc.dram_tensor("w8", [n8, P, E], f8, kind="ExternalInput")
    bias = nc.dram_tensor("bias", [CPC, E], f32, kind="ExternalInput")
    # fp16 copy of the (pre-scaled) bias: the last channel folds its bias
    # in via a K=1 ones-row matmul instead of a VectorE add, so its
    # eviction is a single fused op per PSUM bank (shorter kernel tail)
    bias16 = nc.dram_tensor("bias16", [CPC, E], f16, kind="ExternalInput")
    out = nc.dram_tensor("out", [B, CPC, E], f16, kind="ExternalOutput")

    # DRAM slot index of (c, k) within its dtype-packed tensor
    slot16, slot8 = {}, {}
    i16 = i8 = 0
    for c in range(CPC):
        for k in range(KT):
            if k in K8[c]:
                slot8[(c, k)] = i8
                i8 += 1
            else:
                slot16[(c, k)] = i16
                i16 += 1

    with tile.TileContext(nc) as tc:
        with (
            tc.tile_pool(name="const", bufs=1) as const,
            tc.tile_pool(name="xpool", bufs=2) as xpool,
            tc.tile_pool(name="bpool", bufs=1) as bpool,
            tc.tile_pool(name="bbpool", bufs=2) as bbpool,
            tc.tile_pool(name="ttmp", bufs=8) as ttmp,
            tc.tile_pool(name="wpool", bufs=WBUFS) as wpool,
            tc.tile_pool(name="w8pool", bufs=W8BUFS) as w8pool,
            tc.tile_pool(name="opool", bufs=4) as opool,
            tc.tile_pool(name="psum", bufs=NBT * NET, space="PSUM") as psum,
        ):
            zbias = const.tile([P, 1], f32)
            nc.any.memset(zbias[:], 0.0)
            wz = const.tile([P, P], f16)
            nc.vector.memset(wz[:], 0.0)
            ones = const.tile([1, P], f16)
            nc.vector.memset(ones[:], 1.0)

            xtiles: dict[int, object] = {}
            btiles: dict[int, object] = {}

            def bias_broadcast(c):
                bsb = bpool.tile([1, E], f32, name="bsb")
                nc.gpsimd.dma_start(bsb[:], bias[c : c + 1, :])
                bbc = bbpool.tile([P, E], f32, name="bbc")
                nc.gpsimd.partition_broadcast(bbc[:], bsb[:])
                btiles[c] = bbc

            # Channel 0's x: k0-1 races the first W chunk on the HWDGE
            # rings; the rest rides the otherwise-idle SWDGE ring.
            xsb0 = xpool.tile([P, KT * B], f8, name="xsb")
            nc.sync.dma_start(xsb0[:, : 2 * B], xt8[0, :, : 2 * B])
            nc.gpsimd.dma_start(xsb0[:, 2 * B : 8 * B], xt8[0, :, 2 * B : 8 * B])
            nc.gpsimd.dma_start(xsb0[:, 8 * B :], xt8[0, :, 8 * B :])
            xtiles[0] = xsb0

            def prefetch_channel(c, after):
                # next channel's x slab + bias on the SWDGE ring, held back
                # until mid-channel so it doesn't steal HBM bandwidth from
                # the live W stream (GpSimd is in-order: gating the slab
                # gates everything behind it too)
                if c < 2:
                    xsb = xpool.tile([P, KT * B], f8, name="xsb")
                    xdma = nc.gpsimd.dma_start(xsb[:], xt8[c, :, :])
                else:
                    xsb = xpool.tile([P, KT * B], f16, name="xsb")
                    xdma = nc.gpsimd.dma_start(xsb[:], xt16[c - 2, :, :])
                add_dep_helper(
                    xdma.ins,
                    after.ins,
                    reason="x prefetch waits for mid-channel W chunk",
                )
                xtiles[c] = xsb
                if c == CPC - 1:
                    # last channel: bias rides the matmul (K=1 ones row),
                    # no partition-broadcast copy needed
                    b16 = bpool.tile([1, E], f16, name="b16")
                    nc.gpsimd.dma_start(b16[:], bias16[c : c + 1, :])
                    btiles[c] = b16
                else:
                    bias_broadcast(c)

            htog = [0]
            for c in range(CPC):
                xsb = xtiles[c]
                ps = [
                    [
                        psum.tile([P, FREE], f32, name="ps")
                        for _ in range(NET)
                    ]
                    for _ in range(NBT)
                ]
                if c == 0:
                    # PE warmup: ~4us of zero matmuls into the first PSUM
                    # bank while the first x/W DMAs are in flight, so the
                    # HAM clock gate reaches 8/8 before the real matmuls
                    # start. The real k0 matmul (start=True) overwrites.
                    for _ in range(NWARM):
                        nc.tensor.matmul(
                            ps[0][0][:, :P], wz[:], wz[:], start=True, stop=True
                        )

                last = c == CPC - 1
                if last:
                    # seed each PSUM bank with the bias via a K=1 ones-row
                    # matmul (start=True clears the bank, writes bias[e] to
                    # every batch row); the k-loop then accumulates on top
                    # and eviction needs no separate bias add
                    b16 = btiles[c]
                    for bt in range(NBT):
                        for et in range(NET):
                            nc.tensor.matmul(
                                ps[bt][et][:],
                                ones[:],
                                b16[:, et * FREE : (et + 1) * FREE],
                                start=True,
                                stop=False,
                            )
                prefetched = False
                for ci, (k0, nkt, is8) in enumerate(_chunks(c)):
                    if is8:
                        wsb = w8pool.tile([P, KC, E], f8, name="w8sb")
                        src, slot = w8, slot8[(c, k0)]
                    else:
                        wsb = wpool.tile([P, KC, E], f16, name="wsb")
                        src, slot = w16, slot16[(c, k0)]

                    def span(h0, h1):
                        # DRAM view of k-tiles slot..slot+nkt-1, cols h0:h1
                        return src[slot : slot + nkt, :, h0:h1].rearrange(
                            "k p e -> p k e"
                        )

                    if c == 0 and 6 <= k0 < 8:
                        # ramp k6-k7 ride the SWDGE ring: a third parallel
                        # delivery channel during the bandwidth-starved head
                        wdma = nc.gpsimd.dma_start(wsb[:, :nkt, :], span(0, E))
                        halves = [(0, NET)]
                        if k0 == 7:
                            bias_broadcast(0)
                    else:
                        # E-half split across BOTH HWDGE rings: each half
                        # unlocks its et matmuls as soon as it lands, and
                        # the per-ring FIFO keeps k-tiles in need order
                        htog[0] ^= 1
                        ea = nc.sync if htog[0] else nc.scalar
                        eb = nc.scalar if htog[0] else nc.sync
                        ea.dma_start(wsb[:, :nkt, : E // 2], span(0, E // 2))
                        wdma = eb.dma_start(
                            wsb[:, :nkt, E // 2 :], span(E // 2, E)
                        )
                        halves = [(0, NET // 2), (NET // 2, NET)]

                    for e0, e1 in halves:
                        for kk in range(nkt):
                            k = k0 + kk
                            for bt in range(NBT):
                                lhsT = xsb[
                                    :, k * B + bt * P : k * B + (bt + 1) * P
                                ]
                                for et in range(e0, e1):
                                    nc.tensor.matmul(
                                        ps[bt][et][:],
                                        lhsT,
                                        wsb[:, kk, et * FREE : (et + 1) * FREE],
                                        start=(k == 0 and not last),
                                        stop=(k == KT - 1),
                                    )
                    # gate the next channel's x prefetch behind a
                    # mid-channel W chunk (later on ch0: behind the SWDGE
                    # ramp so the slab doesn't block it)
                    if (
                        not prefetched
                        and k0 + nkt >= (10 if c == 0 else 6)
                        and c + 1 < CPC
                    ):
                        prefetch_channel(c + 1, after=wdma)
                        prefetched = True
                # Evict. Channels 0-2: VectorE adds the broadcast bias
                # (freeing the PSUM bank), ScalarE applies ReLU + 2^-9
                # unscale + fp16 cast. Last channel: bias is already in
                # PSUM, so each bank evicts with ONE fused op, alternating
                # ScalarE (activation) and VectorE (tensor_scalar) so the
                # tail chain runs on two engines in parallel.
                bbc = btiles[c]
                oq = [0]
                for bt in range(NBT):
                    ot = opool.tile([P, E], f16)
                    for et in range(NET):
                        dst = ot[:, et * FREE : (et + 1) * FREE]
                        if last:
                            if (bt * NET + et) % 2 == 0:
                                nc.scalar.activation(
                                    dst,
                                    ps[bt][et][:],
                                    mybir.ActivationFunctionType.Relu,
                                    bias=zbias[:],
                                    scale=1.0 / WSCALE,
                                )
                            else:
                                nc.vector.tensor_scalar(
                                    dst,
                                    ps[bt][et][:],
                                    1.0 / WSCALE,
                                    0.0,
                                    op0=mybir.AluOpType.mult,
                                    op1=mybir.AluOpType.max,
                                )
                        else:
                            tmp = ttmp.tile([P, FREE], f16, name="tmp")
                            nc.vector.tensor_add(
                                tmp[:],
                                ps[bt][et][:],
                                bbc[:, et * FREE : (et + 1) * FREE],
                            )
                            nc.scalar.activation(
                                dst,
                                tmp[:],
                                mybir.ActivationFunctionType.Relu,
                                bias=zbias[:],
                                # fp8-x channels carry an extra 2x in PSUM
                                scale=1.0
                                / (WSCALE * (XSCALE if c < 2 else 1.0)),
                            )
                        if last:
                            # tail: eager per-subtile DMAs spread over all
                            # three rings (the W stream is finished by now)
                            oengs = [
                                nc.gpsimd, nc.gpsimd, nc.sync, nc.scalar,
                                nc.sync, nc.scalar, nc.sync, nc.scalar,
                            ]
                            oeng = oengs[oq[0]]
                            oq[0] += 1
                            oeng.dma_start(
                                out[
                                    bt * P : (bt + 1) * P,
                                    c,
                                    et * FREE : (et + 1) * FREE,
                                ],
                                dst,
                            )
                    if not last:
                        # one 1 MB DMA per (bt, c) on the SWDGE ring,
                        # keeping both HWDGE rings pure-W
                        nc.gpsimd.dma_start(out[bt * P : (bt + 1) * P, c, :], ot[:])
    nc.compile()
    return nc


def _get_nc():
    if "nc" not in _nc_cache:
        _nc_cache["nc"] = _build()
    return _nc_cache["nc"]


def _run(x, W, b, **spmd_kwargs):
    nc = _get_nc()

    in_maps = []
    for i in range(NCORES):
        c0, c1 = i * CPC, (i + 1) * CPC
        # xt[c, p, k*B + b] = x[b, k*P + p, c]; channels 0-1 fp8 (x2)
        xs = x[:, :, c0:c1].astype(np.float32)           # (B, S, CPC)
        xs = xs.transpose(2, 1, 0).reshape(CPC, KT, P, B)
        xs = np.ascontiguousarray(xs.transpose(0, 2, 1, 3)).reshape(
            CPC, P, KT * B
        )
        xt8_i = np.ascontiguousarray(
            (xs[:2] * XSCALE).astype(ml_dtypes.float8_e3m4)
        )
        xt16_i = np.ascontiguousarray(xs[2:].astype(np.float16))
        # dtype-packed k-tiles, value WSCALE * W[c, k*P + p, e]
        ws = (W[c0:c1] * WSCALE).astype(np.float32).reshape(CPC, KT, P, E)
        t16 = [ws[c, k] for c in range(CPC) for k in range(KT)
               if k not in K8[c]]
        t8 = [ws[c, k] for c in range(CPC) for k in range(KT) if k in K8[c]]
        w16_i = np.ascontiguousarray(np.stack(t16).astype(np.float16))
        w8_i = np.ascontiguousarray(
            np.stack(t8).astype(ml_dtypes.float8_e3m4)
        )
        b_i = np.ascontiguousarray((b[c0:c1] * WSCALE).astype(np.float32))
        b16_i = np.ascontiguousarray(b_i.astype(np.float16))
        b_i[:2] *= XSCALE     # match the fp8-x channels' extra PSUM scale
        in_maps.append(
            {
                "xt8": xt8_i,
                "xt16": xt16_i,
                "w16": w16_i,
                "w8": w8_i,
                "bias": b_i,
                "bias16": b16_i,
            }
        )

    res = run_bass_kernel_spmd(
        nc, in_maps, core_ids=list(range(NCORES)), **spmd_kwargs
    )
    out = np.concatenate(
        [r["out"].astype(np.float32) for r in res.results], axis=1
    )
    return out, res


def kernel(x: np.ndarray, W: np.ndarray, b: np.ndarray) -> np.ndarray:
    out, _ = _run(x, W, b)
    return out


# revision 42
# speedup vs baseline: 1.0846x; 1.0846x over previous
"""Grouped per-channel Linear + ReLU on 8 TRN2 NeuronCores.

Problem: out[b,c,e] = relu(sum_s x[b,s,c] * W[c,s,e] + bias[c,e])
  x: (256, 2048, 32) f32, W: (32, 2048, 2048) f32, bias: (32, 2048) f32
  out: (256, 32, 2048) f32

Sharding: expert/channel parallel - core i computes channels [4i, 4i+4).
Each core runs 4 independent GEMMs of (256x2048)@(2048x2048) with the
contraction dim S on SBUF partitions.

Precision: the PE runs fp8 E3M4 at full bf16 rate and the two operand
dtypes of a matmul are independent (both mixed directions verified on
HW). W k-tiles are fp16 or fp8 per a schedule below (40 of 64 fp8,
host pre-scaled by 2^9 into E3M4's normal range); x (stationary
operand) is fp8 on channels 0-1 (pre-scaled by 2) and fp16 on 2-3. The
eviction activation undoes the exact power-of-two scaling. Measured
rel l2 error 1.45e-2 against the 2e-2 gate.

Why mixed precision, and why this placement: two NeuronCores share one
716 GB/s HBM stack, and at kernel start all 8 cores pull their W
streams at once, so the first ~60 us are chip-bandwidth-starved: W
k-tiles arrive slower than the PE's ~294 GB/s consumption, the PE gaps,
and each >3.4 us gap re-throttles the HAM clock gate to 1.2 GHz. The
fp8 k-tiles are therefore concentrated in the contended window -
channel 0 (k2-15), all of channel 1, channel 2 (k10-15) - halving the
bytes exactly where bandwidth is scarce, plus channel 3's tail (k12-15)
which coincides with x-prefetch/output bursts. Steady state is PE-bound
at the 216 ns/matmul roofline (512 N=512 matmuls = 110.5 us/core).

Schedule: W chunks (2 k-tiles; singles for the channel-0 ramp) are
E-half split across BOTH HWDGE rings, so each half unlocks its et
matmuls independently and the PE sees work every ~0.25-0.5 MB of
delivery. Channel-0 k6-7 ride the SWDGE ring (third parallel channel
during the head); ~48 zero warmup matmuls keep the PE busy before the
first data lands so the HAM reaches 8/8 early. x slabs prefetch one
channel ahead on SWDGE, gated behind a mid-channel W chunk; outputs for
channels 0-2 leave on SWDGE; the last channel's leave eagerly per
512-col subtile on all three rings. Eviction: VectorE adds the
broadcast bias, ScalarE applies ReLU + 2^-9 unscale + fp16 cast. The
LAST channel instead seeds its PSUM banks with the bias via K=1
ones-row matmuls, so its eviction is one fused op per bank, alternated
across ScalarE/VectorE - this halves the serial evict chain that forms
the kernel tail after the final matmul.
"""

import os
import sys

for _p in ("/opt/trn_rl_repo", "/root/.axon_site/_ro/trn_rl_repo"):
    if os.path.isdir(_p) and _p not in sys.path:
        sys.path.insert(0, _p)

import numpy as np
import ml_dtypes

import concourse.bacc as bacc
import concourse.mybir as mybir
from concourse import tile
from concourse.bass_utils import run_bass_kernel_spmd
from concourse.tile_rust import add_dep_helper

B, S, C, E = 256, 2048, 32, 2048
NCORES = 8
CPC = C // NCORES          # channels per core = 4
P = 128
KT = S // P                # 16 k-tiles
NBT = B // P               # 2 batch tiles
FREE = 512                 # matmul moving free dim (one PSUM bank of f32)
NET = E // FREE            # 4 e-tiles
KC = 2                     # k-tiles per W DMA chunk
WBUFS = 10                 # fp16 W chunk lookahead
W8BUFS = 12                # fp8 W chunk lookahead
NWARM = 48                 # PE warmup matmuls during the DMA head
WSCALE = 512.0             # host W/bias pre-scale (2^9), undone at evict
XSCALE = 2.0               # extra x pre-scale for the fp8-x channels 0-1

# per-channel fp8 k-tiles: concentrated in the bandwidth-starved first
# ~60us (all of ch0 after the first two tiles, first 10 tiles of ch1)
# plus the channel tails that coincide with prefetch/output bursts.
K8 = {
    0: frozenset(range(2, 16)),
    1: frozenset(range(0, 16)),
    2: frozenset({10, 11, 12, 13, 14, 15}),
    3: frozenset({12, 13, 14, 15}),
}

_nc_cache = {}


def _chunks(c):
    """[(k0, nkt, is8)] covering k 0..15; singles below k8 on channel 0."""
    out = []
    k = 0
    while k < KT:
        is8 = k in K8[c]
        n = 1
        if (
            not (c == 0 and k < 8)
            and k + 1 < KT
            and ((k + 1) in K8[c]) == is8
        ):
            n = KC
        out.append((k, n, is8))
        k += n
    return out


def _build():
    f16 = mybir.dt.float16
    f32 = mybir.dt.float32
    f8 = mybir.dt.float8e3
    nc = bacc.Bacc(None, target_bir_lowering=False)
    # xt[c, p, k*B + b] = x[b, k*P + p, c] : contiguous per partition.
    # Channels 0-1 (the bandwidth-contended head) carry x in fp8 e3m4
    # (scaled by 2 so |x| <= ~11 sits in E3M4's normal range); 2-3 fp16.
    xt8 = nc.dram_tensor("xt8", [2, P, KT * B], f8, kind="ExternalInput")
    xt16 = nc.dram_tensor(
        "xt16", [CPC - 2, P, KT * B], f16, kind="ExternalInput"
    )
    # w16[c, p, j, e] / w8[c, p, j, e]: j-th fp16/fp8 k-tile of channel c
    # (k-tiles packed per dtype in k order), value WSCALE * W[c, k*P+p, e]
    n16 = sum(KT - len(K8[c]) for c in range(CPC))
    n8 = sum(len(K8[c]) for c in range(CPC))
    w16 = nc.dram_tensor("w16", [n16, P, E], f16, kind="ExternalInput")
    w8 = nc.dram_tensor("w8", [n8, P, E], f8, kind="ExternalInput")
    bias = nc.dram_tensor("bias", [CPC, E], f32, kind="ExternalInput")
    # fp16 copy of the (pre-scaled) bias: the last channel folds its bias
    # in via a K=1 ones-row matmul instead of a VectorE add, so its
    # eviction is a single fused op per PSUM bank (shorter kernel tail)
    bias16 = nc.dram_tensor("bias16", [CPC, E], f16, kind="ExternalInput")
    out = nc.dram_tensor("out", [B, CPC, E], f16, kind="ExternalOutput")

    # DRAM slot index of (c, k) within its dtype-packed tensor
    slot16, slot8 = {}, {}
    i16 = i8 = 0
    for c in range(CPC):
        for k in range(KT):
            if k in K8[c]:
                slot8[(c, k)] = i8
                i8 += 1
            else:
                slot16[(c, k)] = i16
                i16 += 1

    with tile.TileContext(nc) as tc:
        with (
            tc.tile_pool(name="const", bufs=1) as const,
            tc.tile_pool(name="xpool", bufs=2) as xpool,
            tc.tile_pool(name="bpool", bufs=1) as bpool,
            tc.tile_pool(name="bbpool", bufs=2) as bbpool,
            tc.tile_pool(name="ttmp", bufs=8) as ttmp,
            tc.tile_pool(name="wpool", bufs=WBUFS) as wpool,
            tc.tile_pool(name="w8pool", bufs=W8BUFS) as w8pool,
            tc.tile_pool(name="opool", bufs=4) as opool,
            tc.tile_pool(name="psum", bufs=NBT * NET, space="PSUM") as psum,
        ):
            zbias = const.tile([P, 1], f32)
            nc.any.memset(zbias[:], 0.0)
            wz = const.tile([P, P], f16)
            nc.vector.memset(wz[:], 0.0)
            ones = const.tile([1, P], f16)
            nc.vector.memset(ones[:], 1.0)

            xtiles: dict[int, object] = {}
            btiles: dict[int, object] = {}

            def bias_broadcast(c):
                bsb = bpool.tile([1, E], f32, name="bsb")
                nc.gpsimd.dma_start(bsb[:], bias[c : c + 1, :])
                bbc = bbpool.tile([P, E], f32, name="bbc")
                nc.gpsimd.partition_broadcast(bbc[:], bsb[:])
                btiles[c] = bbc

            # Channel 0's x: k0-1 races the first W chunk on the HWDGE
            # rings; the rest rides the otherwise-idle SWDGE ring.
            xsb0 = xpool.tile([P, KT * B], f8, name="xsb")
            nc.sync.dma_start(xsb0[:, : 2 * B], xt8[0, :, : 2 * B])
            nc.gpsimd.dma_start(xsb0[:, 2 * B : 8 * B], xt8[0, :, 2 * B : 8 * B])
            nc.gpsimd.dma_start(xsb0[:, 8 * B :], xt8[0, :, 8 * B :])
            xtiles[0] = xsb0

            def prefetch_channel(c, after):
                # next channel's x slab + bias on the SWDGE ring, held back
                # until mid-channel so it doesn't steal HBM bandwidth from
                # the live W stream (GpSimd is in-order: gating the slab
                # gates everything behind it too)
                if c < 2:
                    xsb = xpool.tile([P, KT * B], f8, name="xsb")
                    xdma = nc.gpsimd.dma_start(xsb[:], xt8[c, :, :])
                else:
                    xsb = xpool.tile([P, KT * B], f16, name="xsb")
                    xdma = nc.gpsimd.dma_start(xsb[:], xt16[c - 2, :, :])
                add_dep_helper(
                    xdma.ins,
                    after.ins,
                    reason="x prefetch waits for mid-channel W chunk",
                )
                xtiles[c] = xsb
                if c == CPC - 1:
                    # last channel: bias rides the matmul (K=1 ones row),
                    # no partition-broadcast copy needed
                    b16 = bpool.tile([1, E], f16, name="b16")
                    nc.gpsimd.dma_start(b16[:], bias16[c : c + 1, :])
                    btiles[c] = b16
                else:
                    bias_broadcast(c)

            htog = [0]
            for c in range(CPC):
                xsb = xtiles[c]
                ps = [
                    [
                        psum.tile([P, FREE], f32, name="ps")
                        for _ in range(NET)
                    ]
                    for _ in range(NBT)
                ]
                if c == 0:
                    # PE warmup: ~4us of zero matmuls into the first PSUM
                    # bank while the first x/W DMAs are in flight, so the
                    # HAM clock gate reaches 8/8 before the real matmuls
                    # start. The real k0 matmul (start=True) overwrites.
                    for _ in range(NWARM):
                        nc.tensor.matmul(
                            ps[0][0][:, :P], wz[:], wz[:], start=True, stop=True
                        )

                last = c == CPC - 1
                if last:
                    # seed each PSUM bank with the bias via a K=1 ones-row
                    # matmul (start=True clears the bank, writes bias[e] to
                    # every batch row); the k-loop then accumulates on top
                    # and eviction needs no separate bias add
                    b16 = btiles[c]
                    for bt in range(NBT):
                        for et in range(NET):
                            nc.tensor.matmul(
                                ps[bt][et][:],
                                ones[:],
                                b16[:, et * FREE : (et + 1) * FREE],
                                start=True,
                                stop=False,
                            )
                prefetched = False
                for ci, (k0, nkt, is8) in enumerate(_chunks(c)):
                    if is8:
                        wsb = w8pool.tile([P, KC, E], f8, name="w8sb")
                        src, slot = w8, slot8[(c, k0)]
                    else:
                        wsb = wpool.tile([P, KC, E], f16, name="wsb")
                        src, slot = w16, slot16[(c, k0)]

                    def span(h0, h1):
                        # DRAM view of k-tiles slot..slot+nkt-1, cols h0:h1
                        return src[slot : slot + nkt, :, h0:h1].rearrange(
                            "k p e -> p k e"
                        )

                    if c == 0 and 6 <= k0 < 8:
                        # ramp k6-k7 ride the SWDGE ring: a third parallel
                        # delivery channel during the bandwidth-starved head
                        wdma = nc.gpsimd.dma_start(wsb[:, :nkt, :], span(0, E))
                        halves = [(0, NET)]
                        if k0 == 7:
                            bias_broadcast(0)
                    else:
                        # E-half split across BOTH HWDGE rings: each half
                        # unlocks its et matmuls as soon as it lands, and
                        # the per-ring FIFO keeps k-tiles in need order
                        htog[0] ^= 1
                        ea = nc.sync if htog[0] else nc.scalar
                        eb = nc.scalar if htog[0] else nc.sync
                        ea.dma_start(wsb[:, :nkt, : E // 2], span(0, E // 2))
                        wdma = eb.dma_start(
                            wsb[:, :nkt, E // 2 :], span(E // 2, E)
                        )
                        halves = [(0, NET // 2), (NET // 2, NET)]

                    for e0, e1 in halves:
                        for kk in range(nkt):
                            k = k0 + kk
                            for bt in range(NBT):
                                lhsT = xsb[
                                    :, k * B + bt * P : k * B + (bt + 1) * P
                                ]
                                for et in range(e0, e1):
                                    nc.tensor.matmul(
                                        ps[bt][et][:],
                                        lhsT,
                                        wsb[:, kk, et * FREE : (et + 1) * FREE],
                                        start=(k == 0 and not last),
                                        stop=(k == KT - 1),
                                    )
                    # gate the next channel's x prefetch behind a
                    # mid-channel W chunk (later on ch0: behind the SWDGE
                    # ramp so the slab doesn't block it)
                    if (
                        not prefetched
                        and k0 + nkt >= (10 if c == 0 else 6)
                        and c + 1 < CPC
                    ):
                        prefetch_channel(c + 1, after=wdma)
                        prefetched = True
                # Evict. Channels 0-2: VectorE adds the broadcast bias
                # (freeing the PSUM bank), ScalarE applies ReLU + 2^-9
                # unscale + fp16 cast. Last channel: bias is already in
                # PSUM, so each bank evicts with ONE fused op, alternating
                # ScalarE (activation) and VectorE (tensor_scalar) so the
                # tail chain runs on two engines in parallel.
                bbc = btiles[c]
                oq = [0]
                for bt in range(NBT):
                    ot = opool.tile([P, E], f16)
                    for et in range(NET):
                        dst = ot[:, et * FREE : (et + 1) * FREE]
                        if last:
                            if (bt * NET + et) % 2 == 0:
                                nc.scalar.activation(
                                    dst,
                                    ps[bt][et][:],
                                    mybir.ActivationFunctionType.Relu,
                                    bias=zbias[:],
                                    scale=1.0 / WSCALE,
                                )
                            else:
                                nc.vector.tensor_scalar(
                                    dst,
                                    ps[bt][et][:],
                                    1.0 / WSCALE,
                                    0.0,
                                    op0=mybir.AluOpType.mult,
                                    op1=mybir.AluOpType.max,
                                )
                        else:
                            tmp = ttmp.tile([P, FREE], f16, name="tmp")
                            nc.vector.tensor_add(
                                tmp[:],
                                ps[bt][et][:],
                                bbc[:, et * FREE : (et + 1) * FREE],
                            )
                            nc.scalar.activation(
                                dst,
                                tmp[:],
                                mybir.ActivationFunctionType.Relu,
                                bias=zbias[:],
                                # fp8-x channels carry an extra 2x in PSUM
                                scale=1.0
                                / (WSCALE * (XSCALE if c < 2 else 1.0)),
                            )
                        if last:
                            # tail: eager per-subtile DMAs spread over all
                            # three rings (the W stream is finished by now)
                            oengs = [
                                nc.gpsimd, nc.gpsimd, nc.sync, nc.scalar,
                                nc.sync, nc.scalar, nc.sync, nc.scalar,
                            ]
                            oeng = oengs[oq[0]]
                            oq[0] += 1
                            oeng.dma_start(
                                out[
                                    bt * P : (bt + 1) * P,
                                    c,
                                    et * FREE : (et + 1) * FREE,
                                ],
                                dst,
                            )
                    if not last:
                        # one 1 MB DMA per (bt, c) on the SWDGE ring,
                        # keeping both HWDGE rings pure-W
                        nc.gpsimd.dma_start(out[bt * P : (bt + 1) * P, c, :], ot[:])
    nc.compile()
    return nc


def _get_nc():
    if "nc" not in _nc_cache:
        _nc_cache["nc"] = _build()
    return _nc_cache["nc"]


def _run(x, W, b, **spmd_kwargs):
    nc = _get_nc()

    in_maps = []
    for i in range(NCORES):
        c0, c1 = i * CPC, (i + 1) * CPC
        # xt[c, p, k*B + b] = x[b, k*P + p, c]; channels 0-1 fp8 (x2)
        xs = x[:, :, c0:c1].astype(np.float32)           # (B, S, CPC)
        xs = xs.transpose(2, 1, 0).reshape(CPC, KT, P, B)
        xs = np.ascontiguousarray(xs.transpose(0, 2, 1, 3)).reshape(
            CPC, P, KT * B
        )
        xt8_i = np.ascontiguousarray(
            (xs[:2] * XSCALE).astype(ml_dtypes.float8_e3m4)
        )
        xt16_i = np.ascontiguousarray(xs[2:].astype(np.float16))
        # dtype-packed k-tiles, value WSCALE * W[c, k*P + p, e]
        ws = (W[c0:c1] * WSCALE).astype(np.float32).reshape(CPC, KT, P, E)
        t16 = [ws[c, k] for c in range(CPC) for k in range(KT)
               if k not in K8[c]]
        t8 = [ws[c, k] for c in range(CPC) for k in range(KT) if k in K8[c]]
        w16_i = np.ascontiguousarray(np.stack(t16).astype(np.float16))
        w8_i = np.ascontiguousarray(
            np.stack(t8).astype(ml_dtypes.float8_e3m4)
        )
        b_i = np.ascontiguousarray((b[c0:c1] * WSCALE).astype(np.float32))
        b16_i = np.ascontiguousarray(b_i.astype(np.float16))
        b_i[:2] *= XSCALE     # match the fp8-x channels' extra PSUM scale
        in_maps.append(
            {
                "xt8": xt8_i,
                "xt16": xt16_i,
                "w16": w16_i,
                "w8": w8_i,
                "bias": b_i,
                "bias16": b16_i,
            }
        )

    res = run_bass_kernel_spmd(
        nc, in_maps, core_ids=list(range(NCORES)), **spmd_kwargs
    )
    out = np.concatenate(
        [r["out"].astype(np.float32) for r in res.results], axis=1
    )
    return out, res


def kernel(x: np.ndarray, W: np.ndarray, b: np.ndarray) -> np.ndarray:
    out, _ = _run(x, W, b)
    return out
